# revision 2
# baseline (speedup 1.0000x reference)
"""Trainium2 Bass kernel v2: 7x7 single-channel 2D convolution (zero-padded),
data-parallel over 8 NeuronCores (8 images per core), bf16 matmul path.

Per image (H=W=512, k=7, pad=3), the H-direction convolution for kernel
column dx is a banded matmul over a 128-row window of the padded image:
  psum[m, j] += sum_r bands[r, dx, m] * Xp[r0+r, j+dx],
  bands[r, dx, m] = k[r-m, dx] (0 <= r-m < 7)
Each 128-row window yields M=122 output rows; 5 windows (stride 122)
cover all 512 output rows (last window: K=30 rows -> M=24 outputs).
The 7 dx shifts accumulate into one PSUM bank (full bank [128,512] f32),
then DVE evacuates to SBUF with a cast to bf16 and the ACT ring DMAs to
DRAM. Inputs are pre-padded + converted to bf16 on host; outputs return
as bf16 and are upcast on host (tolerance is 2e-2; bf16 error ~0.4%).
"""

import numpy as np
import ml_dtypes

BF16 = ml_dtypes.bfloat16

B = 64          # total images
NC = 8          # neuron cores
BPC = B // NC   # images per core
H = W = 512
KS = 7
PAD = KS // 2
PADHW = H + 2 * PAD          # 518
MWIN = 122                   # output rows per full window (128 - KS + 1)
NWIN = 5                     # windows per image (4 full + 1 short)
F32 = np.float32


def _host_prep(X, kern):
    """Returns per-core padded bf16 image arrays and the band tensor."""
    Xb = X[:, 0].astype(BF16)
    xp = np.zeros((B, PADHW, PADHW), dtype=BF16)
    xp[:, PAD:PAD + H, PAD:PAD + W] = Xb
    xs = [np.ascontiguousarray(xp[c * BPC:(c + 1) * BPC]) for c in range(NC)]
    # bands[r, dx, m] = kern[r - m, dx] for 0 <= r - m < 7
    bands = np.zeros((128, KS, 128), dtype=BF16)
    kb = kern.astype(BF16)
    for dy in range(KS):
        r = np.arange(dy, 128)
        m = r - dy
        bands[r, :, m] = kb[dy, :]
    return xs, bands


def build_bass(split_waits=True):
    from concourse import bass, mybir
    from concourse import tile

    bf = mybir.dt.bfloat16
    f32 = mybir.dt.float32
    nc = bass.Bass("TRN2", target_bir_lowering=False, debug=False)

    xpad_d = nc.dram_tensor("xpad", [BPC, PADHW, PADHW], bf, kind="ExternalInput")
    bands_d = nc.dram_tensor("bands", [128, KS, 128], bf, kind="ExternalInput")
    y_d = nc.dram_tensor("y", [BPC, H, W], bf, kind="ExternalOutput")

    with tile.TileContext(nc) as tc:
        with (
            tc.tile_pool(name="const", bufs=1) as const_pool,
            tc.tile_pool(name="win", bufs=6) as win_pool,
            tc.tile_pool(name="ps", bufs=8, space=bass.MemorySpace.PSUM) as psum_pool,
            tc.tile_pool(name="st", bufs=4) as stage_pool,
        ):
            bands_sb = const_pool.tile([128, KS, 128], bf, name="bands_sb")
            nc.sync.dma_start(out=bands_sb[:], in_=bands_d[:])

            for b in range(BPC):
                for w in range(NWIN):
                    r0 = MWIN * w                      # padded-row start
                    K = 128 if w < NWIN - 1 else PADHW - r0   # 128 or 30
                    M = K - KS + 1                     # 122 or 24
                    win_t = win_pool.tile([128, PADHW], bf, name="win", tag="win")
                    nc.sync.dma_start(
                        out=win_t[0:K, :], in_=xpad_d[b, r0:r0 + K, :]
                    )
                    ps = psum_pool.tile([128, W], f32, name="ps", tag="ps")
                    for dx in range(KS):
                        nc.tensor.matmul(
                            ps[0:M, :],
                            bands_sb[0:K, dx, 0:M],
                            win_t[0:K, dx:dx + W],
                            start=(dx == 0),
                            stop=(dx == KS - 1),
                        )
                    stage = stage_pool.tile([128, W], bf, name="st", tag="st")
                    nc.vector.tensor_copy(stage[0:M, :], ps[0:M, :])
                    # output DMAs ride the ACT HWDGE ring (input DMAs own
                    # the SP ring)
                    nc.scalar.dma_start(
                        out=y_d[b, r0:r0 + M, :], in_=stage[0:M, :]
                    )
    if split_waits:
        _split_multi_waits(nc, mybir)
    return nc


def _split_multi_waits(nc, mybir):
    """This walrus build accepts at most one semaphore wait per
    instruction; Tile can emit several. Hoist all but the last wait onto
    NoOps inserted just before, on the same engine queue (engine programs
    preserve relative instruction order, so the waits still gate the
    original instruction)."""
    uid = 0
    for fn in nc.m.functions:
        for blk in fn.blocks:
            insts = blk.instructions
            out = []
            for ins in insts:
                si = getattr(ins, "sync_info", None)
                if si is not None and len(si.on_wait) > 1:
                    waits = list(si.on_wait)
                    for wt in waits[:-1]:
                        nop = mybir.InstNoOp(
                            name=f"waitnop_{uid}", engine=ins.engine
                        )
                        nop.sync_info = mybir.SyncInfo(on_wait=[wt], on_update=[])
                        out.append(nop)
                        uid += 1
                    ins.sync_info = mybir.SyncInfo(
                        on_wait=[waits[-1]], on_update=list(si.on_update)
                    )
                out.append(ins)
            blk.instructions = out
    return nc


_CACHED = {}


def kernel(X, kernel):
    X = np.ascontiguousarray(np.asarray(X), dtype=F32)
    kern = np.asarray(kernel, dtype=F32)
    assert X.shape == (B, 1, H, W), X.shape
    assert kern.shape == (KS, KS), kern.shape

    from concourse.bass_utils import run_bass_kernel_spmd

    if "nc" not in _CACHED:
        _CACHED["nc"] = build_bass()
    nc = _CACHED["nc"]

    xs, bands = _host_prep(X, kern)
    in_maps = [{"xpad": xs[c], "bands": bands} for c in range(NC)]
    res = run_bass_kernel_spmd(nc, in_maps, list(range(NC)))
    out = np.empty((B, 1, H, W), dtype=F32)
    for c in range(NC):
        out[c * BPC:(c + 1) * BPC, 0] = res.results[c]["y"].astype(F32)
    return out
